# revision 26
# baseline (speedup 1.0000x reference)
"""Trainium2 Bass kernel for nn_ComputeDistances (vq_codebook).

dist[k, m] = || X @ (M[:, m] - c_k) ||_2,  X:[4096,512], M:[512,4096], C:[2048,512]

Reformulated via the Gram matrix G = X^T X (512x512):
    dist^2[k, m] = m^T G m  -  2 c_k^T G m  +  c_k^T G c_k

Sharding: 8 cores as a 2(K) x 4(m) grid; each core computes its
[1024, 1024] output slab independently (no collectives).

All heavy matmuls are fp8e4 DoubleRow (2 fp8 rows per PE pass: a
contraction-512 product needs 2 instructions instead of 4). Measured on
HW: one FD-512 DR matmul streams in ~216ns with LDWEIGHTS hidden.

fp8 range/precision handling:
  - G's diagonal (~4096) would dominate fp8 quantization error, so stage
    A subtracts 4096*I on the PE (one (-64I)^T(64I) matmul per diag
    block) and stages B/B2 restore it with a +128*m-hat correction
    DoubleRow matmul (idp = [128*I; 0] pairs) inside each PSUM group.
  - H = G@M is cast to fp8 as H/256 on the ACT engine; sqXM reduces
    hf8 .* m8 on the Pool engine (all SBUF - GPSIMD cannot touch PSUM),
    sqXC reduces the f32 PSUM on DVE.
  - Host-side rows (nam/nac) cancel the per-query component of the fp8
    rounding of m and c.
  - sqXM/sqXC fold into stage C's PSUM via one contraction-2 matmul.

Scheduling against the HAM clock-gate: the PE must stay busy or the
clock drops to 1.2GHz and stays there. Stage A is upper-triangular
(mirrored via fp8 PE transposes), and stage C is split by m-halves:
C(s=0) interleaves with B(s=1) so the sqrt/DMA stream of the first half
hides under matmul work. All output DMAs issue from the SP queue - a
dma_start costs ~600ns of sequencer time and must not serialize with
the ACT sqrts.

Scale ledger (P* = PSUM value):
  A:  P_G  = G - 4096 I          gxx8 = P_G/32           (fp8)
  B:  P_H  = gxx8@m8 + 128 m8 = H/32
      hf8  = P_H/8 = H/256 (ACT)   p16 = hf8 .* m8 = Hm/256 (Pool)
      sqm  = ones^T p16 = sqXM/256   wwt1 = sqm + nam (via tiny DMA)
  B2: P_W  = gxx8@c8 + 128 c8 = GC2/32   (c8 = -2C^T)
      q16  = P_W .* c8 = c(Gc)/8 (DVE)
      sqc  = (ones/32)^T q16 = sqXC/256  vvt0 = sqc + nac
  C:  P_D  = c8^T @ hf8 + vvt^T wwt = dist^2/256
      out  = Sqrt(256 * P_D)  (ACT, fp16; host upcasts to f32)
"""

import os
import numpy as np

N, D, M_COLS, K = 4096, 512, 4096, 2048
N_CORES = 8
KC, MC = 2, 4  # core grid: K-split x M-split
K_LOC, M_LOC = K // KC, M_COLS // MC  # 1024, 1024

P = 128
XT_N = 8           # X tiles of 512 rows (2 DoubleRow groups each)
XT_R = 4           # sub-rows per partition per X tile
DC = D // P        # 4 contraction chunks over D
MS = M_LOC // 512  # 2 m-slices of 512
KS = K_LOC // 512  # 2 k-slices of 512
KT = K_LOC // P    # 8 k-tiles
WARM_MMS = 20

_compiled = {}


def _build_nc():
    import concourse.mybir as mybir
    import concourse.tile as tile
    from concourse import bacc
    from concourse.masks import make_identity

    f32 = mybir.dt.float32
    f16 = mybir.dt.float16
    bf16 = mybir.dt.bfloat16
    f8 = mybir.dt.float8e4
    DR = mybir.MatmulPerfMode.DoubleRow
    MULT = mybir.AluOpType.mult
    ADD = mybir.AluOpType.add

    nc = bacc.Bacc("TRN2", target_bir_lowering=False, debug=False)

    # host-packed flat layouts: one contiguous span per partition
    x_d = nc.dram_tensor("x", [P, XT_N * XT_R * D], f8, kind="ExternalInput")
    m_d = nc.dram_tensor("m8", [P, DC * M_LOC], f8, kind="ExternalInput")
    c_d = nc.dram_tensor("c8", [P, DC * K_LOC], f8, kind="ExternalInput")
    nam_d = nc.dram_tensor("nam", [1, M_LOC], f16, kind="ExternalInput")
    nac_d = nc.dram_tensor("nac", [1, K_LOC], f16, kind="ExternalInput")
    o_d = nc.dram_tensor("out", [K_LOC, M_LOC], f16, kind="ExternalOutput")

    with tile.TileContext(nc) as tc:
        with (
            tc.tile_pool(name="xp", bufs=1) as xp,
            tc.tile_pool(name="res", bufs=1) as res,
            tc.tile_pool(name="wk", bufs=1) as wk,
            tc.tile_pool(name="op", bufs=6) as op,
            tc.tile_pool(name="psA", bufs=4, space="PSUM") as psA,
            tc.tile_pool(name="psS", bufs=1, space="PSUM") as psS,
        ):
            # ---- PE warmup: tiny bf16 matmuls on zero tiles (no input deps) ----
            wl = res.tile([P, 1], bf16, tag="wl")
            wz = res.tile([P, P], bf16, tag="wz")
            nc.vector.memset(wl[:], 0.0)
            nc.vector.memset(wz[:], 0.0)
            wps = psS.tile([1, P], mybir.dt.float32, tag="sqm0")
            for _ in range(WARM_MMS):
                nc.tensor.matmul(wps[:], wl[:], wz[:], start=True, stop=True)

            # ---- input loads: X on both HWDGE queues, then m8/c8 ----
            dma_engs = [nc.sync, nc.scalar]
            xq = []
            for g in range(XT_N):
                t = xp.tile([P, XT_R, D], f8, tag=f"xq{g}", name=f"xq{g}")
                dma_engs[g % 2].dma_start(
                    t[:], x_d.ap()[:, g * XT_R * D : (g + 1) * XT_R * D]
                )
                xq.append(t)
            # m/c inputs ride the software DGE so the HWDGE queues carry X only
            ms8 = res.tile([P, DC, M_LOC], f8, tag="ms8")
            ct8 = res.tile([P, DC, K_LOC], f8, tag="ct8")
            nc.gpsimd.dma_start(ct8[:], c_d.ap())
            nc.gpsimd.dma_start(ms8[:], m_d.ap())
            nam = res.tile([1, M_LOC], f16, tag="nam")
            nac = res.tile([1, K_LOC], f16, tag="nac")
            nc.gpsimd.dma_start(nam[:], nam_d.ap())
            nc.gpsimd.dma_start(nac[:], nac_d.ap())

            # ---- constants ----
            ones16 = res.tile([P, P], f16, tag="ones16")
            nc.vector.memset(ones16[:], 1.0)
            ones32 = res.tile([P, P], f16, tag="ones32")
            nc.vector.memset(ones32[:], 1.0 / 32.0)
            idf = res.tile([P, P], f32, tag="idf")
            make_identity(nc, idf[:])
            id8 = res.tile([P, P], f8, tag="id8")
            nc.vector.tensor_scalar_mul(id8[:], idf[:], 1.0)
            wneg = res.tile([P, P], bf16, tag="wneg")
            nc.vector.tensor_scalar_mul(wneg[:], idf[:], -64.0)
            wpos = res.tile([P, P], bf16, tag="wpos")
            nc.vector.tensor_scalar_mul(wpos[:], idf[:], 64.0)
            # idp[q] = DoubleRow pair [128*I at sub-slot q, 0 elsewhere]
            idp = []
            for q in range(2):
                t = res.tile([P, 2, P], f8, tag=f"idp{q}")
                nc.vector.memset(t[:], 0.0)
                nc.vector.tensor_scalar_mul(t[:, q], idf[:], 128.0)
                idp.append(t)
            # rank-2 fold tiles: vvt = [sqxc;1], wwt = [1;sqxm] (row 1 of wwt
            # is DMA-filled since vector engines cannot write partition 1)
            vvt = res.tile([2, K_LOC], f16, tag="vvt")
            nc.vector.memset(vvt[:], 1.0)
            wwt = res.tile([2, M_LOC], f16, tag="wwt")
            nc.vector.memset(wwt[:], 1.0)
            sqxm16 = res.tile([1, M_LOC], f16, tag="sqxm16")

            # resident intermediates
            gxx8 = res.tile([P, DC, D], f8, tag="gxx8")    # (G - 4096 I)/32
            hf8 = res.tile([P, DC, M_LOC], f8, tag="hf8")  # H/256
            q16t = res.tile([P, DC, K_LOC], f16, tag="q16t")   # P_W .* c8
            p16t = res.tile([P, DC, M_LOC], f16, tag="p16t")   # hf8 .* m8

            # ---- stage A: upper-tri G = X^T X - 4096 I ----
            ptags = ["sqm0", "sqm1", "sqc0", "sqc1"]
            pgs = [
                psS.tile([P, 512 - 128 * t], mybir.dt.float32, tag=ptags[t],
                         name=f"pgA{t}")
                for t in range(DC)
            ]
            first = True
            for g in range(XT_N):
                xt = xq[g]
                for f in range(0, XT_R, 2):
                    for t in range(DC):
                        nc.tensor.matmul(
                            pgs[t][:],
                            xt[:, f : f + 2, t * P : (t + 1) * P],
                            xt[:, f : f + 2, t * P :],
                            start=first,
                            stop=(g == XT_N - 1 and f == XT_R - 2),
                            perf_mode=DR,
                        )
                    if first:
                        first = False
                        for t in range(DC):
                            nc.tensor.matmul(
                                pgs[t][:, :P],
                                wneg[:],
                                wpos[:],
                                start=False,
                                stop=False,
                                skip_group_check=True,
                            )
            # diag copies: DVE + ACT split
            for c in range(DC):
                eng = nc.vector.tensor_scalar_mul if c % 2 == 0 else nc.scalar.mul
                eng(gxx8[:, c, c * P :], pgs[c][:], 1.0 / 32.0)

            def emit_mirrors():
                # fp8 transpose mode requires output element step of 2
                for t in range(DC):
                    for c in range(t + 1, DC):
                        tp = psA.tile([P, P, 2], f8, tag="ph")
                        nc.tensor.transpose(
                            tp[:, :, 0], gxx8[:, t, c * P : (c + 1) * P], id8[:]
                        )
                        nc.vector.tensor_copy(gxx8[:, c, t * P : (t + 1) * P],
                                              tp[:, :, 0])

            sqc = [
                psS.tile([P, 512], mybir.dt.float32, tag=f"sqc{s}", name=f"sqc{s}")
                for s in range(KS)
            ]
            sqm = [
                psS.tile([P, 512], mybir.dt.float32, tag=f"sqm{s}", name=f"sqm{s}")
                for s in range(MS)
            ]

            def emit_B2(t, s):
                j2, q2 = t // 2, t % 2
                ph = psA.tile([P, 512], mybir.dt.float32, tag="ph")
                for j in range(2):
                    nc.tensor.matmul(
                        ph[:],
                        gxx8[:, 2 * j : 2 * j + 2, t * P : (t + 1) * P],
                        ct8[:, 2 * j : 2 * j + 2, s * 512 : (s + 1) * 512],
                        start=(j == 0),
                        stop=False,
                        perf_mode=DR,
                    )
                nc.tensor.matmul(
                    ph[:],
                    idp[q2][:],
                    ct8[:, 2 * j2 : 2 * j2 + 2, s * 512 : (s + 1) * 512],
                    start=False,
                    stop=True,
                    perf_mode=DR,
                )
                nc.vector.tensor_tensor(
                    q16t[:, t, s * 512 : (s + 1) * 512],
                    ph[:],
                    ct8[:, t, s * 512 : (s + 1) * 512],
                    MULT,
                )

            def emit_B(t, s):
                j2, q2 = t // 2, t % 2
                ph = psA.tile([P, 512], mybir.dt.float32, tag="ph")
                for j in range(2):
                    nc.tensor.matmul(
                        ph[:],
                        gxx8[:, 2 * j : 2 * j + 2, t * P : (t + 1) * P],
                        ms8[:, 2 * j : 2 * j + 2, s * 512 : (s + 1) * 512],
                        start=(j == 0),
                        stop=False,
                        perf_mode=DR,
                    )
                nc.tensor.matmul(
                    ph[:],
                    idp[q2][:],
                    ms8[:, 2 * j2 : 2 * j2 + 2, s * 512 : (s + 1) * 512],
                    start=False,
                    stop=True,
                    perf_mode=DR,
                )
                # s=1 casts on DVE: the ACT queue is sqrt-busy during C(s=0)
                if s == 0:
                    nc.scalar.mul(hf8[:, t, s * 512 : (s + 1) * 512], ph[:], 0.125)
                else:
                    nc.vector.tensor_scalar_mul(
                        hf8[:, t, s * 512 : (s + 1) * 512], ph[:], 0.125
                    )
                nc.gpsimd.tensor_tensor(
                    p16t[:, t, s * 512 : (s + 1) * 512],
                    hf8[:, t, s * 512 : (s + 1) * 512],
                    ms8[:, t, s * 512 : (s + 1) * 512],
                    MULT,
                )

            def emit_C(kt, s):
                pgc = psA.tile([P, 512], mybir.dt.float32, tag="ph")
                for j in range(2):
                    nc.tensor.matmul(
                        pgc[:],
                        ct8[:, 2 * j : 2 * j + 2, kt * P : (kt + 1) * P],
                        hf8[:, 2 * j : 2 * j + 2, s * 512 : (s + 1) * 512],
                        start=(j == 0),
                        stop=False,
                        perf_mode=DR,
                    )
                nc.tensor.matmul(
                    pgc[:],
                    vvt[:, kt * P : (kt + 1) * P],
                    wwt[:, s * 512 : (s + 1) * 512],
                    start=False,
                    stop=True,
                )
                ob = op.tile([P, 512], f16, tag="ob")
                nc.scalar.activation(
                    ob[:], pgc[:], mybir.ActivationFunctionType.Sqrt, scale=256.0
                )
                nc.sync.dma_start(
                    o_d.ap()[kt * P : (kt + 1) * P, s * 512 : (s + 1) * 512],
                    ob[:],
                )

            # ---- B2 (t=3 first, mirrors overlap), sqc reduction, vvt ----
            emit_B2(DC - 1, 0)
            emit_B2(DC - 1, 1)
            emit_mirrors()
            for t in range(DC - 2, -1, -1):
                emit_B2(t, 0)
                emit_B2(t, 1)
            for idx, t in enumerate(range(DC - 1, -1, -1)):
                for s in range(KS):
                    nc.tensor.matmul(
                        sqc[s][:],
                        ones32[:],
                        q16t[:, t, s * 512 : (s + 1) * 512],
                        start=(idx == 0),
                        stop=(idx == DC - 1),
                    )
            for s in range(KS):
                nc.vector.tensor_tensor(
                    vvt[0:1, s * 512 : (s + 1) * 512],
                    sqc[s][0:1, :],
                    nac[0:1, s * 512 : (s + 1) * 512],
                    ADD,
                )

            # ---- B(s=0), sqm[0], wwt half ----
            for t in range(DC - 1, -1, -1):
                emit_B(t, 0)
            for idx, t in enumerate(range(DC - 1, -1, -1)):
                nc.tensor.matmul(
                    sqm[0][:],
                    ones16[:],
                    p16t[:, t, 0:512],
                    start=(idx == 0),
                    stop=(idx == DC - 1),
                )
            nc.vector.tensor_tensor(
                sqxm16[0:1, 0:512], sqm[0][0:1, :], nam[0:1, 0:512], ADD
            )
            nc.sync.dma_start(wwt[1:2, 0:512], sqxm16[0:1, 0:512])

            # ---- C(s=0) interleaved with B(s=1), front-loaded, to keep the
            # PE dense through the sqrt stream ----
            for kt in range(KT):
                emit_C(kt, 0)
                if 1 <= kt <= DC:
                    emit_B(DC - kt, 1)
                elif kt == DC + 1:
                    for idx, t in enumerate(range(DC - 1, -1, -1)):
                        nc.tensor.matmul(
                            sqm[1][:],
                            ones16[:],
                            p16t[:, t, 512:1024],
                            start=(idx == 0),
                            stop=(idx == DC - 1),
                        )
                elif kt == DC + 2:
                    nc.vector.tensor_tensor(
                        sqxm16[0:1, 512:1024], sqm[1][0:1, :],
                        nam[0:1, 512:1024], ADD
                    )
                    nc.sync.dma_start(wwt[1:2, 512:1024], sqxm16[0:1, 512:1024])

            # ---- C(s=1) ----
            for kt in range(KT):
                emit_C(kt, 1)

    nc.compile()
    return nc


def _get_nc():
    if "nc" not in _compiled:
        _compiled["nc"] = _build_nc()
    return _compiled["nc"]


def _make_in_maps(X, Mf, C):
    import concourse.mybir as mybir

    np8 = mybir.dt.np(mybir.dt.float8e4)
    x8 = np.ascontiguousarray(X).astype(np8)
    # pack rows g*(P*XT_R) + p*XT_R + f -> [p, (g*XT_R + f)*512 + d]
    xp8 = np.ascontiguousarray(
        x8.reshape(XT_N, P, XT_R, D).transpose(1, 0, 2, 3)
        .reshape(P, XT_N * XT_R * D)
    )
    in_maps = []
    for core in range(N_CORES):
        kc, mc = divmod(core, MC)
        Mslab = Mf[:, mc * M_LOC : (mc + 1) * M_LOC]
        Cslab = C[kc * K_LOC : (kc + 1) * K_LOC, :]
        m8 = np.ascontiguousarray(Mslab).astype(np8)
        c8 = np.ascontiguousarray(-2.0 * Cslab.T).astype(np8)
        # pack rows c*128 + p -> [p, c*cols + j]
        m8p = np.ascontiguousarray(
            m8.reshape(DC, P, M_LOC).transpose(1, 0, 2).reshape(P, DC * M_LOC)
        )
        c8p = np.ascontiguousarray(
            c8.reshape(DC, P, K_LOC).transpose(1, 0, 2).reshape(P, DC * K_LOC)
        )
        # corrections for the Gram-diagonal term: computed dist^2 uses the
        # fp8-rounded m-hat/c-hat; subtract 4096*(2<v,dv>+|dv|^2) per query
        dmv = m8.astype(np.float32) - Mslab
        dcv = c8.astype(np.float32) / -2.0 - Cslab.T
        am = 4096.0 * (2.0 * np.einsum("dm,dm->m", Mslab, dmv)
                       + np.einsum("dm,dm->m", dmv, dmv))
        ac = 4096.0 * (2.0 * np.einsum("dk,dk->k", Cslab.T, dcv)
                       + np.einsum("dk,dk->k", dcv, dcv))
        nam = np.ascontiguousarray(-am[None, :] / 256.0).astype(np.float16)
        nac = np.ascontiguousarray(-ac[None, :] / 256.0).astype(np.float16)
        in_maps.append({"x": xp8, "m8": m8p, "c8": c8p, "nam": nam, "nac": nac})
    return in_maps


def _extract_out(raw):
    return np.asarray(raw).astype(np.float32)


def kernel(in_activations, M, centroids):
    from concourse import bass_utils

    X = np.asarray(in_activations, dtype=np.float32)
    Mf = np.asarray(M, dtype=np.float32)
    C = np.asarray(centroids, dtype=np.float32)

    nc = _get_nc()
    in_maps = _make_in_maps(X, Mf, C)

    res = bass_utils.run_bass_kernel_spmd(
        nc,
        in_maps,
        core_ids=list(range(N_CORES)),
        trace=bool(int(os.environ.get("KERNEL_TRACE", "0"))),
    )
    if res.exec_time_ns is not None:
        print(f"HW exec time: {res.exec_time_ns} ns")
        _compiled["exec_time_ns"] = res.exec_time_ns

    out = np.empty((K, M_COLS), dtype=np.float32)
    for core in range(N_CORES):
        kc, mc = divmod(core, MC)
        out[kc * K_LOC : (kc + 1) * K_LOC, mc * M_LOC : (mc + 1) * M_LOC] = (
            _extract_out(res.results[core]["out"])
        )
    return out


# revision 32
# speedup vs baseline: 1.0278x; 1.0278x over previous
"""Trainium2 Bass kernel for nn_ComputeDistances (vq_codebook).

dist[k, m] = || X @ (M[:, m] - c_k) ||_2,  X:[4096,512], M:[512,4096], C:[2048,512]

Reformulated via the Gram matrix G = X^T X (512x512):
    dist^2[k, m] = m^T G m  -  2 c_k^T G m  +  c_k^T G c_k

Sharding: 8 cores as a 2(K) x 4(m) grid; each core computes its
[1024, 1024] output slab independently (no collectives).

All heavy matmuls are fp8e4 DoubleRow (2 fp8 rows per PE pass: a
contraction-512 product needs 2 instructions instead of 4). Measured on
HW: one FD-512 DR matmul streams in ~216ns with LDWEIGHTS hidden.

fp8 range/precision handling:
  - G's diagonal (~4096) would dominate fp8 quantization error, so stage
    A subtracts 4096*I on the PE (one (-64I)^T(64I) matmul per diag
    block) and stages B/B2 restore it with a +128*m-hat correction
    DoubleRow matmul (idp = [128*I; 0] pairs) inside each PSUM group.
  - H = G@M is cast to fp8 as H/256 on the ACT engine; sqXM reduces
    hf8 .* m8 on the Pool engine (all SBUF - GPSIMD cannot touch PSUM),
    sqXC reduces the f32 PSUM on DVE.
  - Host-side rows (nam/nac) cancel the per-query component of the fp8
    rounding of m and c.
  - sqXM/sqXC fold into stage C's PSUM via one contraction-2 matmul.

Scheduling against the HAM clock-gate: the PE must stay busy or the
clock drops to 1.2GHz and stays there. Stage A is upper-triangular
(mirrored via fp8 PE transposes), and stage C is split by m-halves:
C(s=0) interleaves with B(s=1) so the sqrt/DMA stream of the first half
hides under matmul work. All output DMAs issue from the SP queue - a
dma_start costs ~600ns of sequencer time and must not serialize with
the ACT sqrts.

Scale ledger (P* = PSUM value):
  A:  P_G  = G - 4096 I          gxx8 = P_G/32           (fp8)
  B:  P_H  = gxx8@m8 + 128 m8 = H/32
      hf8  = P_H/8 = H/256 (ACT)   p16 = hf8 .* m8 = Hm/256 (Pool)
      sqm  = ones^T p16 = sqXM/256   wwt1 = sqm + nam (via tiny DMA)
  B2: P_W  = gxx8@c8 + 128 c8 = GC2/32   (c8 = -2C^T)
      q16  = P_W .* c8 = c(Gc)/8 (DVE)
      sqc  = (ones/32)^T q16 = sqXC/256  vvt0 = sqc + nac
  C:  P_D  = c8^T @ hf8 + vvt^T wwt = dist^2/256
      out  = Sqrt(256 * P_D)  (ACT, fp16; host upcasts to f32)
"""

import os
import numpy as np

N, D, M_COLS, K = 4096, 512, 4096, 2048
N_CORES = 8
KC, MC = 2, 4  # core grid: K-split x M-split
K_LOC, M_LOC = K // KC, M_COLS // MC  # 1024, 1024

P = 128
XT_N = 8           # X tiles of 512 rows (2 DoubleRow groups each)
XT_R = 4           # sub-rows per partition per X tile
DC = D // P        # 4 contraction chunks over D
MS = M_LOC // 512  # 2 m-slices of 512
KS = K_LOC // 512  # 2 k-slices of 512
KT = K_LOC // P    # 8 k-tiles
WARM_MMS = 32

_compiled = {}


def _build_nc():
    import concourse.mybir as mybir
    import concourse.tile as tile
    from concourse import bacc
    from concourse.masks import make_identity

    f32 = mybir.dt.float32
    f16 = mybir.dt.float16
    bf16 = mybir.dt.bfloat16
    f8 = mybir.dt.float8e4
    DR = mybir.MatmulPerfMode.DoubleRow
    MULT = mybir.AluOpType.mult
    ADD = mybir.AluOpType.add

    nc = bacc.Bacc("TRN2", target_bir_lowering=False, debug=False)

    # host-packed flat layouts: one contiguous span per partition
    x_d = nc.dram_tensor("x", [P, XT_N * XT_R * D], f8, kind="ExternalInput")
    m_d = nc.dram_tensor("m8", [P, DC * M_LOC], f8, kind="ExternalInput")
    c_d = nc.dram_tensor("c8", [P, DC * K_LOC], f8, kind="ExternalInput")
    nam_d = nc.dram_tensor("nam", [1, M_LOC], f16, kind="ExternalInput")
    nac_d = nc.dram_tensor("nac", [1, K_LOC], f16, kind="ExternalInput")
    o_d = nc.dram_tensor("out", [K_LOC, M_LOC], f16, kind="ExternalOutput")

    with tile.TileContext(nc) as tc:
        with (
            tc.tile_pool(name="xp", bufs=1) as xp,
            tc.tile_pool(name="res", bufs=1) as res,
            tc.tile_pool(name="wk", bufs=1) as wk,
            tc.tile_pool(name="op", bufs=6) as op,
            tc.tile_pool(name="psA", bufs=4, space="PSUM") as psA,
            tc.tile_pool(name="psS", bufs=1, space="PSUM") as psS,
        ):
            # ---- PE warmup: tiny bf16 matmuls on zero tiles (no input deps) ----
            wl = res.tile([P, 1], bf16, tag="wl")
            wz = res.tile([P, P], bf16, tag="wz")
            nc.vector.memset(wl[:], 0.0)
            nc.vector.memset(wz[:], 0.0)
            wps = psS.tile([1, P], mybir.dt.float32, tag="sqm0")
            for _ in range(WARM_MMS):
                nc.tensor.matmul(wps[:], wl[:], wz[:], start=True, stop=True)

            # ---- input loads: X on both HWDGE queues, then m8/c8 ----
            dma_engs = [nc.sync, nc.scalar]
            xq = []
            for g in range(XT_N):
                t = xp.tile([P, XT_R, D], f8, tag=f"xq{g}", name=f"xq{g}")
                dma_engs[g % 2].dma_start(
                    t[:], x_d.ap()[:, g * XT_R * D : (g + 1) * XT_R * D]
                )
                xq.append(t)
            # m/c inputs ride the software DGE so the HWDGE queues carry X only
            ms8 = res.tile([P, DC, M_LOC], f8, tag="ms8")
            ct8 = res.tile([P, DC, K_LOC], f8, tag="ct8")
            nc.gpsimd.dma_start(ct8[:], c_d.ap())
            nc.gpsimd.dma_start(ms8[:], m_d.ap())
            nam = res.tile([1, M_LOC], f16, tag="nam")
            nac = res.tile([1, K_LOC], f16, tag="nac")
            nc.gpsimd.dma_start(nam[:], nam_d.ap())
            nc.gpsimd.dma_start(nac[:], nac_d.ap())

            # ---- constants ----
            ones16 = res.tile([P, P], f16, tag="ones16")
            nc.vector.memset(ones16[:], 1.0)
            ones32 = res.tile([P, P], f16, tag="ones32")
            nc.vector.memset(ones32[:], 1.0 / 32.0)
            idf = res.tile([P, P], f32, tag="idf")
            make_identity(nc, idf[:])
            id8 = res.tile([P, P], f8, tag="id8")
            nc.vector.tensor_scalar_mul(id8[:], idf[:], 1.0)
            wneg = res.tile([P, P], bf16, tag="wneg")
            nc.vector.tensor_scalar_mul(wneg[:], idf[:], -64.0)
            wpos = res.tile([P, P], bf16, tag="wpos")
            nc.vector.tensor_scalar_mul(wpos[:], idf[:], 64.0)
            # idp[q] = DoubleRow pair [128*I at sub-slot q, 0 elsewhere]
            idp = []
            for q in range(2):
                t = res.tile([P, 2, P], f8, tag=f"idp{q}")
                nc.vector.memset(t[:], 0.0)
                nc.vector.tensor_scalar_mul(t[:, q], idf[:], 128.0)
                idp.append(t)
            # rank-2 fold tiles: vvt = [1;sqxc], wwt = [sqxm;1]. Vector engines
            # cannot write partition 1, so the DMA-filled row is the EARLY
            # sqXC one (latency hides under stage B); sqXM lands in wwt row 0
            # via a plain DVE write right before its first use.
            vvt = res.tile([2, K_LOC], f16, tag="vvt")
            nc.vector.memset(vvt[:], 1.0)
            wwt = res.tile([2, M_LOC], f16, tag="wwt")
            nc.vector.memset(wwt[:], 1.0)
            sqxc16 = res.tile([1, K_LOC], f16, tag="sqxc16")

            # resident intermediates
            gxx8 = res.tile([P, DC, D], f8, tag="gxx8")    # (G - 4096 I)/32
            hf8 = res.tile([P, DC, M_LOC], f8, tag="hf8")  # H/256
            q16t = res.tile([P, DC, K_LOC], f16, tag="q16t")   # P_W .* c8
            p16t = res.tile([P, DC, M_LOC], f16, tag="p16t")   # hf8 .* m8

            # ---- stage A: upper-tri G = X^T X - 4096 I ----
            ptags = ["sqm0", "sqm1", "sqc0", "sqc1"]
            pgs = [
                psS.tile([P, 512 - 128 * t], mybir.dt.float32, tag=ptags[t],
                         name=f"pgA{t}")
                for t in range(DC)
            ]
            first = True
            for g in range(XT_N):
                xt = xq[g]
                for f in range(0, XT_R, 2):
                    for t in range(DC):
                        nc.tensor.matmul(
                            pgs[t][:],
                            xt[:, f : f + 2, t * P : (t + 1) * P],
                            xt[:, f : f + 2, t * P :],
                            start=first,
                            stop=(g == XT_N - 1 and f == XT_R - 2),
                            perf_mode=DR,
                        )
                    if first:
                        first = False
                        for t in range(DC):
                            nc.tensor.matmul(
                                pgs[t][:, :P],
                                wneg[:],
                                wpos[:],
                                start=False,
                                stop=False,
                                skip_group_check=True,
                            )
            # diag copies: DVE + ACT split
            for c in range(DC):
                eng = nc.vector.tensor_scalar_mul if c % 2 == 0 else nc.scalar.mul
                eng(gxx8[:, c, c * P :], pgs[c][:], 1.0 / 32.0)

            def emit_mirrors():
                # fp8 transpose mode requires output element step of 2
                for t in range(DC):
                    for c in range(t + 1, DC):
                        tp = psA.tile([P, P, 2], f8, tag="ph")
                        nc.tensor.transpose(
                            tp[:, :, 0], gxx8[:, t, c * P : (c + 1) * P], id8[:]
                        )
                        nc.vector.tensor_copy(gxx8[:, c, t * P : (t + 1) * P],
                                              tp[:, :, 0])

            sqc = [
                psS.tile([P, 512], mybir.dt.float32, tag=f"sqc{s}", name=f"sqc{s}")
                for s in range(KS)
            ]
            sqm = [
                psS.tile([P, 512], mybir.dt.float32, tag=f"sqm{s}", name=f"sqm{s}")
                for s in range(MS)
            ]

            def emit_B2(t, s):
                j2, q2 = t // 2, t % 2
                ph = psA.tile([P, 512], mybir.dt.float32, tag="ph")
                for j in range(2):
                    nc.tensor.matmul(
                        ph[:],
                        gxx8[:, 2 * j : 2 * j + 2, t * P : (t + 1) * P],
                        ct8[:, 2 * j : 2 * j + 2, s * 512 : (s + 1) * 512],
                        start=(j == 0),
                        stop=False,
                        perf_mode=DR,
                    )
                nc.tensor.matmul(
                    ph[:],
                    idp[q2][:],
                    ct8[:, 2 * j2 : 2 * j2 + 2, s * 512 : (s + 1) * 512],
                    start=False,
                    stop=True,
                    perf_mode=DR,
                )
                nc.vector.tensor_tensor(
                    q16t[:, t, s * 512 : (s + 1) * 512],
                    ph[:],
                    ct8[:, t, s * 512 : (s + 1) * 512],
                    MULT,
                )

            def emit_B(t, s):
                j2, q2 = t // 2, t % 2
                ph = psA.tile([P, 512], mybir.dt.float32, tag="ph")
                for j in range(2):
                    nc.tensor.matmul(
                        ph[:],
                        gxx8[:, 2 * j : 2 * j + 2, t * P : (t + 1) * P],
                        ms8[:, 2 * j : 2 * j + 2, s * 512 : (s + 1) * 512],
                        start=(j == 0),
                        stop=False,
                        perf_mode=DR,
                    )
                nc.tensor.matmul(
                    ph[:],
                    idp[q2][:],
                    ms8[:, 2 * j2 : 2 * j2 + 2, s * 512 : (s + 1) * 512],
                    start=False,
                    stop=True,
                    perf_mode=DR,
                )
                # s=1 casts on DVE: the ACT queue is sqrt-busy during C(s=0)
                if s == 0:
                    nc.scalar.mul(hf8[:, t, s * 512 : (s + 1) * 512], ph[:], 0.125)
                else:
                    nc.vector.tensor_scalar_mul(
                        hf8[:, t, s * 512 : (s + 1) * 512], ph[:], 0.125
                    )
                nc.gpsimd.tensor_tensor(
                    p16t[:, t, s * 512 : (s + 1) * 512],
                    hf8[:, t, s * 512 : (s + 1) * 512],
                    ms8[:, t, s * 512 : (s + 1) * 512],
                    MULT,
                )

            def emit_C(kt, s):
                pgc = psA.tile([P, 512], mybir.dt.float32, tag="ph")
                for j in range(2):
                    nc.tensor.matmul(
                        pgc[:],
                        ct8[:, 2 * j : 2 * j + 2, kt * P : (kt + 1) * P],
                        hf8[:, 2 * j : 2 * j + 2, s * 512 : (s + 1) * 512],
                        start=(j == 0),
                        stop=False,
                        perf_mode=DR,
                    )
                nc.tensor.matmul(
                    pgc[:],
                    vvt[:, kt * P : (kt + 1) * P],
                    wwt[:, s * 512 : (s + 1) * 512],
                    start=False,
                    stop=True,
                )
                ob = op.tile([P, 512], f16, tag="ob")
                nc.scalar.activation(
                    ob[:], pgc[:], mybir.ActivationFunctionType.Sqrt, scale=256.0
                )
                (nc.sync if kt % 2 == 0 else nc.gpsimd).dma_start(
                    o_d.ap()[kt * P : (kt + 1) * P, s * 512 : (s + 1) * 512],
                    ob[:],
                )

            # ---- B2 (t=3 first, mirrors overlap), sqc reduction, vvt ----
            emit_B2(DC - 1, 0)
            emit_B2(DC - 1, 1)
            emit_mirrors()
            for t in range(DC - 2, -1, -1):
                emit_B2(t, 0)
                emit_B2(t, 1)
            for idx, t in enumerate(range(DC - 1, -1, -1)):
                for s in range(KS):
                    nc.tensor.matmul(
                        sqc[s][:],
                        ones32[:],
                        q16t[:, t, s * 512 : (s + 1) * 512],
                        start=(idx == 0),
                        stop=(idx == DC - 1),
                    )
            for s in range(KS):
                nc.vector.tensor_tensor(
                    sqxc16[0:1, s * 512 : (s + 1) * 512],
                    sqc[s][0:1, :],
                    nac[0:1, s * 512 : (s + 1) * 512],
                    ADD,
                )
            nc.sync.dma_start(vvt[1:2, :], sqxc16[0:1, :])

            # ---- B(s=0), sqm[0], wwt half ----
            for t in range(DC - 1, -1, -1):
                emit_B(t, 0)
            for idx, t in enumerate(range(DC - 1, -1, -1)):
                nc.tensor.matmul(
                    sqm[0][:],
                    ones16[:],
                    p16t[:, t, 0:512],
                    start=(idx == 0),
                    stop=(idx == DC - 1),
                )
            nc.vector.tensor_tensor(
                wwt[0:1, 0:512], sqm[0][0:1, :], nam[0:1, 0:512], ADD
            )

            # ---- C(s=0) interleaved with B(s=1), front-loaded, to keep the
            # PE dense through the sqrt stream ----
            for kt in range(KT):
                emit_C(kt, 0)
                if 1 <= kt <= DC:
                    emit_B(DC - kt, 1)
                elif kt == DC + 1:
                    for idx, t in enumerate(range(DC - 1, -1, -1)):
                        nc.tensor.matmul(
                            sqm[1][:],
                            ones16[:],
                            p16t[:, t, 512:1024],
                            start=(idx == 0),
                            stop=(idx == DC - 1),
                        )
                elif kt == DC + 2:
                    nc.vector.tensor_tensor(
                        wwt[0:1, 512:1024], sqm[1][0:1, :],
                        nam[0:1, 512:1024], ADD
                    )

            # ---- C(s=1) ----
            for kt in range(KT):
                emit_C(kt, 1)

    nc.compile()
    return nc


def _get_nc():
    if "nc" not in _compiled:
        _compiled["nc"] = _build_nc()
    return _compiled["nc"]


def _make_in_maps(X, Mf, C):
    import concourse.mybir as mybir

    np8 = mybir.dt.np(mybir.dt.float8e4)
    x8 = np.ascontiguousarray(X).astype(np8)
    # pack rows g*(P*XT_R) + p*XT_R + f -> [p, (g*XT_R + f)*512 + d]
    xp8 = np.ascontiguousarray(
        x8.reshape(XT_N, P, XT_R, D).transpose(1, 0, 2, 3)
        .reshape(P, XT_N * XT_R * D)
    )
    in_maps = []
    for core in range(N_CORES):
        kc, mc = divmod(core, MC)
        Mslab = Mf[:, mc * M_LOC : (mc + 1) * M_LOC]
        Cslab = C[kc * K_LOC : (kc + 1) * K_LOC, :]
        m8 = np.ascontiguousarray(Mslab).astype(np8)
        c8 = np.ascontiguousarray(-2.0 * Cslab.T).astype(np8)
        # pack rows c*128 + p -> [p, c*cols + j]
        m8p = np.ascontiguousarray(
            m8.reshape(DC, P, M_LOC).transpose(1, 0, 2).reshape(P, DC * M_LOC)
        )
        c8p = np.ascontiguousarray(
            c8.reshape(DC, P, K_LOC).transpose(1, 0, 2).reshape(P, DC * K_LOC)
        )
        # corrections for the Gram-diagonal term: computed dist^2 uses the
        # fp8-rounded m-hat/c-hat; subtract 4096*(2<v,dv>+|dv|^2) per query
        dmv = m8.astype(np.float32) - Mslab
        dcv = c8.astype(np.float32) / -2.0 - Cslab.T
        am = 4096.0 * (2.0 * np.einsum("dm,dm->m", Mslab, dmv)
                       + np.einsum("dm,dm->m", dmv, dmv))
        ac = 4096.0 * (2.0 * np.einsum("dk,dk->k", Cslab.T, dcv)
                       + np.einsum("dk,dk->k", dcv, dcv))
        nam = np.ascontiguousarray(-am[None, :] / 256.0).astype(np.float16)
        nac = np.ascontiguousarray(-ac[None, :] / 256.0).astype(np.float16)
        in_maps.append({"x": xp8, "m8": m8p, "c8": c8p, "nam": nam, "nac": nac})
    return in_maps


def _extract_out(raw):
    return np.asarray(raw).astype(np.float32)


def kernel(in_activations, M, centroids):
    from concourse import bass_utils

    X = np.asarray(in_activations, dtype=np.float32)
    Mf = np.asarray(M, dtype=np.float32)
    C = np.asarray(centroids, dtype=np.float32)

    nc = _get_nc()
    in_maps = _make_in_maps(X, Mf, C)

    res = bass_utils.run_bass_kernel_spmd(
        nc,
        in_maps,
        core_ids=list(range(N_CORES)),
        trace=bool(int(os.environ.get("KERNEL_TRACE", "0"))),
    )
    if res.exec_time_ns is not None:
        print(f"HW exec time: {res.exec_time_ns} ns")
        _compiled["exec_time_ns"] = res.exec_time_ns

    out = np.empty((K, M_COLS), dtype=np.float32)
    for core in range(N_CORES):
        kc, mc = divmod(core, MC)
        out[kc * K_LOC : (kc + 1) * K_LOC, mc * M_LOC : (mc + 1) * M_LOC] = (
            _extract_out(res.results[core]["out"])
        )
    return out


# revision 35
# speedup vs baseline: 1.0979x; 1.0682x over previous
"""Trainium2 Bass kernel for nn_ComputeDistances (vq_codebook).

dist[k, m] = || X @ (M[:, m] - c_k) ||_2,  X:[4096,512], M:[512,4096], C:[2048,512]

Reformulated via the Gram matrix G = X^T X (512x512):
    dist^2[k, m] = m^T G m  -  2 c_k^T G m  +  c_k^T G c_k

Sharding: 8 cores as a 2(K) x 4(m) grid; each core computes its
[1024, 1024] output slab independently (no collectives).

All heavy matmuls are fp8e4 DoubleRow (2 fp8 rows per PE pass: a
contraction-512 product needs 2 instructions instead of 4). Measured on
HW: one FD-512 DR matmul streams in ~216ns with LDWEIGHTS hidden.

fp8 range/precision handling:
  - G's diagonal (~4096) would dominate fp8 quantization error, so stage
    A subtracts 4096*I on the PE (one (-64I)^T(64I) matmul per diag
    block) and stages B/B2 restore it with a +128*m-hat correction
    DoubleRow matmul (idp = [128*I; 0] pairs) inside each PSUM group.
  - H = G@M is cast to fp8 as H/256 on the ACT engine; sqXM reduces
    hf8 .* m8 on the Pool engine (all SBUF - GPSIMD cannot touch PSUM),
    sqXC reduces the f32 PSUM on DVE.
  - Host-side rows (nam/nac) cancel the per-query component of the fp8
    rounding of m and c.
  - sqXM/sqXC fold into stage C's PSUM via one contraction-2 matmul.

Scheduling against the HAM clock-gate: the PE must stay busy or the
clock drops to 1.2GHz and stays there. Stage A is upper-triangular
(mirrored via fp8 PE transposes), and stage C is split by m-halves:
C(s=0) interleaves with B(s=1) so the sqrt/DMA stream of the first half
hides under matmul work. All output DMAs issue from the SP queue - a
dma_start costs ~600ns of sequencer time and must not serialize with
the ACT sqrts.

Scale ledger (P* = PSUM value):
  A:  P_G  = G - 4096 I          gxx8 = P_G/32           (fp8)
  B:  P_H  = gxx8@m8 + 128 m8 = H/32
      hf8  = P_H/8 = H/256 (ACT)   p16 = hf8 .* m8 = Hm/256 (Pool)
      sqm  = ones^T p16 = sqXM/256   wwt1 = sqm + nam (via tiny DMA)
  B2: P_W  = gxx8@c8 + 128 c8 = GC2/32   (c8 = -2C^T)
      q16  = P_W .* c8 = c(Gc)/8 (DVE)
      sqc  = (ones/32)^T q16 = sqXC/256  vvt0 = sqc + nac
  C:  P_D  = c8^T @ hf8 + vvt^T wwt = dist^2/256
      out  = Sqrt(256 * P_D)  (ACT, fp16; host upcasts to f32)
"""

import os
import numpy as np

N, D, M_COLS, K = 4096, 512, 4096, 2048
N_CORES = 8
KC, MC = 2, 4  # core grid: K-split x M-split
K_LOC, M_LOC = K // KC, M_COLS // MC  # 1024, 1024

P = 128
XT_N = 8           # X tiles of 512 rows (2 DoubleRow groups each)
XT_R = 4           # sub-rows per partition per X tile
DC = D // P        # 4 contraction chunks over D
MS = M_LOC // 512  # 2 m-slices of 512
KS = K_LOC // 512  # 2 k-slices of 512
KT = K_LOC // P    # 8 k-tiles
WARM_MMS = 28

_compiled = {}


def _build_nc():
    import concourse.mybir as mybir
    import concourse.tile as tile
    from concourse import bacc
    from concourse.masks import make_identity

    f32 = mybir.dt.float32
    f16 = mybir.dt.float16
    bf16 = mybir.dt.bfloat16
    f8 = mybir.dt.float8e4
    DR = mybir.MatmulPerfMode.DoubleRow
    MULT = mybir.AluOpType.mult
    ADD = mybir.AluOpType.add

    nc = bacc.Bacc("TRN2", target_bir_lowering=False, debug=False)

    # host-packed flat layouts: one contiguous span per partition
    x_d = nc.dram_tensor("x", [P, XT_N * XT_R * D], f8, kind="ExternalInput")
    m_d = nc.dram_tensor("m8", [P, DC * M_LOC], f8, kind="ExternalInput")
    c_d = nc.dram_tensor("c8", [P, DC * K_LOC], f8, kind="ExternalInput")
    nam_d = nc.dram_tensor("nam", [1, M_LOC], f16, kind="ExternalInput")
    nac_d = nc.dram_tensor("nac", [1, K_LOC], f16, kind="ExternalInput")
    o_d = nc.dram_tensor("out", [K_LOC, M_LOC], f16, kind="ExternalOutput")

    with tile.TileContext(nc) as tc:
        with (
            tc.tile_pool(name="xp", bufs=1) as xp,
            tc.tile_pool(name="res", bufs=1) as res,
            tc.tile_pool(name="wk", bufs=1) as wk,
            tc.tile_pool(name="op", bufs=6) as op,
            tc.tile_pool(name="psA", bufs=4, space="PSUM") as psA,
            tc.tile_pool(name="psS", bufs=1, space="PSUM") as psS,
        ):
            # ---- PE warmup: tiny bf16 matmuls on zero tiles (no input deps) ----
            wl = res.tile([P, 1], bf16, tag="wl")
            wz = res.tile([P, P], bf16, tag="wz")
            nc.vector.memset(wl[:], 0.0)
            nc.vector.memset(wz[:], 0.0)
            wps = psS.tile([1, P], mybir.dt.float32, tag="sqm0")
            for _ in range(WARM_MMS):
                nc.tensor.matmul(wps[:], wl[:], wz[:], start=True, stop=True)

            # ---- input loads: X on both HWDGE queues, then m8/c8 ----
            dma_engs = [nc.sync, nc.scalar]
            xq = []
            for g in range(XT_N):
                t = xp.tile([P, XT_R, D], f8, tag=f"xq{g}", name=f"xq{g}")
                dma_engs[g % 2].dma_start(
                    t[:], x_d.ap()[:, g * XT_R * D : (g + 1) * XT_R * D]
                )
                xq.append(t)
            # m/c inputs follow X on the HWDGE queues (not needed until ~23us)
            ms8 = res.tile([P, DC, M_LOC], f8, tag="ms8")
            ct8 = res.tile([P, DC, K_LOC], f8, tag="ct8")
            nc.scalar.dma_start(ct8[:], c_d.ap())
            nc.sync.dma_start(ms8[:], m_d.ap())
            nam = res.tile([1, M_LOC], f16, tag="nam")
            nac = res.tile([1, K_LOC], f16, tag="nac")
            nc.sync.dma_start(nam[:], nam_d.ap())
            nc.scalar.dma_start(nac[:], nac_d.ap())

            # ---- constants ----
            ones16 = res.tile([P, P], f16, tag="ones16")
            nc.vector.memset(ones16[:], 1.0)
            ones32 = res.tile([P, P], f16, tag="ones32")
            nc.vector.memset(ones32[:], 1.0 / 32.0)
            idf = res.tile([P, P], f32, tag="idf")
            make_identity(nc, idf[:])
            id8 = res.tile([P, P], f8, tag="id8")
            nc.vector.tensor_scalar_mul(id8[:], idf[:], 1.0)
            wneg = res.tile([P, P], bf16, tag="wneg")
            nc.vector.tensor_scalar_mul(wneg[:], idf[:], -64.0)
            wpos = res.tile([P, P], bf16, tag="wpos")
            nc.vector.tensor_scalar_mul(wpos[:], idf[:], 64.0)
            # idp[q] = DoubleRow pair [128*I at sub-slot q, 0 elsewhere]
            idp = []
            for q in range(2):
                t = res.tile([P, 2, P], f8, tag=f"idp{q}")
                nc.vector.memset(t[:], 0.0)
                nc.vector.tensor_scalar_mul(t[:, q], idf[:], 128.0)
                idp.append(t)
            # rank-2 fold tiles: vvt = [1;sqxc], wwt = [sqxm;1]. Vector engines
            # cannot write partition 1, so the DMA-filled row is the EARLY
            # sqXC one (latency hides under stage B); sqXM lands in wwt row 0
            # via a plain DVE write right before its first use.
            vvt = res.tile([2, K_LOC], f16, tag="vvt")
            nc.vector.memset(vvt[:], 1.0)
            wwt = res.tile([2, M_LOC], f16, tag="wwt")
            nc.vector.memset(wwt[:], 1.0)
            sqxc16 = res.tile([1, K_LOC], f16, tag="sqxc16")

            # resident intermediates
            gxx8 = res.tile([P, DC, D], f8, tag="gxx8")    # (G - 4096 I)/32
            hf8 = res.tile([P, DC, M_LOC], f8, tag="hf8")  # H/256
            q16t = res.tile([P, DC, K_LOC], f16, tag="q16t")   # P_W .* c8
            p16t = res.tile([P, DC, M_LOC], f16, tag="p16t")   # hf8 .* m8

            # ---- stage A: upper-tri G = X^T X - 4096 I ----
            ptags = ["sqm0", "sqm1", "sqc0", "sqc1"]
            pgs = [
                psS.tile([P, 512 - 128 * t], mybir.dt.float32, tag=ptags[t],
                         name=f"pgA{t}")
                for t in range(DC)
            ]
            first = True
            for g in range(XT_N):
                xt = xq[g]
                for f in range(0, XT_R, 2):
                    for t in range(DC):
                        nc.tensor.matmul(
                            pgs[t][:],
                            xt[:, f : f + 2, t * P : (t + 1) * P],
                            xt[:, f : f + 2, t * P :],
                            start=first,
                            stop=(g == XT_N - 1 and f == XT_R - 2),
                            perf_mode=DR,
                        )
                    if first:
                        first = False
                        for t in range(DC):
                            nc.tensor.matmul(
                                pgs[t][:, :P],
                                wneg[:],
                                wpos[:],
                                start=False,
                                stop=False,
                                skip_group_check=True,
                            )
            # diag copies: DVE + ACT split
            for c in range(DC):
                eng = nc.vector.tensor_scalar_mul if c % 2 == 0 else nc.scalar.mul
                eng(gxx8[:, c, c * P :], pgs[c][:], 1.0 / 32.0)

            def emit_mirrors():
                # fp8 transpose mode requires output element step of 2
                for t in range(DC):
                    for c in range(t + 1, DC):
                        tp = psA.tile([P, P, 2], f8, tag="ph")
                        nc.tensor.transpose(
                            tp[:, :, 0], gxx8[:, t, c * P : (c + 1) * P], id8[:]
                        )
                        nc.vector.tensor_copy(gxx8[:, c, t * P : (t + 1) * P],
                                              tp[:, :, 0])

            sqc = [
                psS.tile([P, 512], mybir.dt.float32, tag=f"sqc{s}", name=f"sqc{s}")
                for s in range(KS)
            ]
            sqm = [
                psS.tile([P, 512], mybir.dt.float32, tag=f"sqm{s}", name=f"sqm{s}")
                for s in range(MS)
            ]

            def emit_B2(t, s):
                j2, q2 = t // 2, t % 2
                ph = psA.tile([P, 512], mybir.dt.float32, tag="ph")
                for j in range(2):
                    nc.tensor.matmul(
                        ph[:],
                        gxx8[:, 2 * j : 2 * j + 2, t * P : (t + 1) * P],
                        ct8[:, 2 * j : 2 * j + 2, s * 512 : (s + 1) * 512],
                        start=(j == 0),
                        stop=False,
                        perf_mode=DR,
                    )
                nc.tensor.matmul(
                    ph[:],
                    idp[q2][:],
                    ct8[:, 2 * j2 : 2 * j2 + 2, s * 512 : (s + 1) * 512],
                    start=False,
                    stop=True,
                    perf_mode=DR,
                )
                nc.vector.tensor_tensor(
                    q16t[:, t, s * 512 : (s + 1) * 512],
                    ph[:],
                    ct8[:, t, s * 512 : (s + 1) * 512],
                    MULT,
                )

            def emit_B(t, s):
                j2, q2 = t // 2, t % 2
                ph = psA.tile([P, 512], mybir.dt.float32, tag="ph")
                for j in range(2):
                    nc.tensor.matmul(
                        ph[:],
                        gxx8[:, 2 * j : 2 * j + 2, t * P : (t + 1) * P],
                        ms8[:, 2 * j : 2 * j + 2, s * 512 : (s + 1) * 512],
                        start=(j == 0),
                        stop=False,
                        perf_mode=DR,
                    )
                nc.tensor.matmul(
                    ph[:],
                    idp[q2][:],
                    ms8[:, 2 * j2 : 2 * j2 + 2, s * 512 : (s + 1) * 512],
                    start=False,
                    stop=True,
                    perf_mode=DR,
                )
                # s=1 casts on DVE: the ACT queue is sqrt-busy during C(s=0)
                if s == 0:
                    nc.scalar.mul(hf8[:, t, s * 512 : (s + 1) * 512], ph[:], 0.125)
                else:
                    nc.vector.tensor_scalar_mul(
                        hf8[:, t, s * 512 : (s + 1) * 512], ph[:], 0.125
                    )
                nc.gpsimd.tensor_tensor(
                    p16t[:, t, s * 512 : (s + 1) * 512],
                    hf8[:, t, s * 512 : (s + 1) * 512],
                    ms8[:, t, s * 512 : (s + 1) * 512],
                    MULT,
                )

            def emit_C(kt, s):
                pgc = psA.tile([P, 512], mybir.dt.float32, tag="ph")
                for j in range(2):
                    nc.tensor.matmul(
                        pgc[:],
                        ct8[:, 2 * j : 2 * j + 2, kt * P : (kt + 1) * P],
                        hf8[:, 2 * j : 2 * j + 2, s * 512 : (s + 1) * 512],
                        start=(j == 0),
                        stop=False,
                        perf_mode=DR,
                    )
                nc.tensor.matmul(
                    pgc[:],
                    vvt[:, kt * P : (kt + 1) * P],
                    wwt[:, s * 512 : (s + 1) * 512],
                    start=False,
                    stop=True,
                )
                ob = op.tile([P, 512], f16, tag="ob")
                nc.scalar.activation(
                    ob[:], pgc[:], mybir.ActivationFunctionType.Sqrt, scale=256.0
                )
                nc.sync.dma_start(
                    o_d.ap()[kt * P : (kt + 1) * P, s * 512 : (s + 1) * 512],
                    ob[:],
                )

            # ---- B2 (t=3 first, mirrors overlap), sqc reduction, vvt ----
            emit_B2(DC - 1, 0)
            emit_B2(DC - 1, 1)
            emit_mirrors()
            for t in range(DC - 2, -1, -1):
                emit_B2(t, 0)
                emit_B2(t, 1)
            for idx, t in enumerate(range(DC - 1, -1, -1)):
                for s in range(KS):
                    nc.tensor.matmul(
                        sqc[s][:],
                        ones32[:],
                        q16t[:, t, s * 512 : (s + 1) * 512],
                        start=(idx == 0),
                        stop=(idx == DC - 1),
                    )
            for s in range(KS):
                nc.vector.tensor_tensor(
                    sqxc16[0:1, s * 512 : (s + 1) * 512],
                    sqc[s][0:1, :],
                    nac[0:1, s * 512 : (s + 1) * 512],
                    ADD,
                )
            nc.sync.dma_start(vvt[1:2, :], sqxc16[0:1, :])

            # ---- B(s=0), sqm[0], wwt half ----
            for t in range(DC - 1, -1, -1):
                emit_B(t, 0)
            for idx, t in enumerate(range(DC - 1, -1, -1)):
                nc.tensor.matmul(
                    sqm[0][:],
                    ones16[:],
                    p16t[:, t, 0:512],
                    start=(idx == 0),
                    stop=(idx == DC - 1),
                )
            nc.vector.tensor_tensor(
                wwt[0:1, 0:512], sqm[0][0:1, :], nam[0:1, 0:512], ADD
            )

            # ---- C(s=0) interleaved with B(s=1), front-loaded, to keep the
            # PE dense through the sqrt stream ----
            for kt in range(KT):
                emit_C(kt, 0)
                if 1 <= kt <= DC:
                    emit_B(DC - kt, 1)
                elif kt == DC + 1:
                    for idx, t in enumerate(range(DC - 1, -1, -1)):
                        nc.tensor.matmul(
                            sqm[1][:],
                            ones16[:],
                            p16t[:, t, 512:1024],
                            start=(idx == 0),
                            stop=(idx == DC - 1),
                        )
                elif kt == DC + 2:
                    nc.vector.tensor_tensor(
                        wwt[0:1, 512:1024], sqm[1][0:1, :],
                        nam[0:1, 512:1024], ADD
                    )

            # ---- C(s=1) ----
            for kt in range(KT):
                emit_C(kt, 1)

    nc.compile()
    return nc


def _get_nc():
    if "nc" not in _compiled:
        _compiled["nc"] = _build_nc()
    return _compiled["nc"]


def _make_in_maps(X, Mf, C):
    import concourse.mybir as mybir

    np8 = mybir.dt.np(mybir.dt.float8e4)
    x8 = np.ascontiguousarray(X).astype(np8)
    # pack rows g*(P*XT_R) + p*XT_R + f -> [p, (g*XT_R + f)*512 + d]
    xp8 = np.ascontiguousarray(
        x8.reshape(XT_N, P, XT_R, D).transpose(1, 0, 2, 3)
        .reshape(P, XT_N * XT_R * D)
    )
    in_maps = []
    for core in range(N_CORES):
        kc, mc = divmod(core, MC)
        Mslab = Mf[:, mc * M_LOC : (mc + 1) * M_LOC]
        Cslab = C[kc * K_LOC : (kc + 1) * K_LOC, :]
        m8 = np.ascontiguousarray(Mslab).astype(np8)
        c8 = np.ascontiguousarray(-2.0 * Cslab.T).astype(np8)
        # pack rows c*128 + p -> [p, c*cols + j]
        m8p = np.ascontiguousarray(
            m8.reshape(DC, P, M_LOC).transpose(1, 0, 2).reshape(P, DC * M_LOC)
        )
        c8p = np.ascontiguousarray(
            c8.reshape(DC, P, K_LOC).transpose(1, 0, 2).reshape(P, DC * K_LOC)
        )
        # corrections for the Gram-diagonal term: computed dist^2 uses the
        # fp8-rounded m-hat/c-hat; subtract 4096*(2<v,dv>+|dv|^2) per query
        dmv = m8.astype(np.float32) - Mslab
        dcv = c8.astype(np.float32) / -2.0 - Cslab.T
        am = 4096.0 * (2.0 * np.einsum("dm,dm->m", Mslab, dmv)
                       + np.einsum("dm,dm->m", dmv, dmv))
        ac = 4096.0 * (2.0 * np.einsum("dk,dk->k", Cslab.T, dcv)
                       + np.einsum("dk,dk->k", dcv, dcv))
        nam = np.ascontiguousarray(-am[None, :] / 256.0).astype(np.float16)
        nac = np.ascontiguousarray(-ac[None, :] / 256.0).astype(np.float16)
        in_maps.append({"x": xp8, "m8": m8p, "c8": c8p, "nam": nam, "nac": nac})
    return in_maps


def _extract_out(raw):
    return np.asarray(raw).astype(np.float32)


def kernel(in_activations, M, centroids):
    from concourse import bass_utils

    X = np.asarray(in_activations, dtype=np.float32)
    Mf = np.asarray(M, dtype=np.float32)
    C = np.asarray(centroids, dtype=np.float32)

    nc = _get_nc()
    in_maps = _make_in_maps(X, Mf, C)

    res = bass_utils.run_bass_kernel_spmd(
        nc,
        in_maps,
        core_ids=list(range(N_CORES)),
        trace=bool(int(os.environ.get("KERNEL_TRACE", "0"))),
    )
    if res.exec_time_ns is not None:
        print(f"HW exec time: {res.exec_time_ns} ns")
        _compiled["exec_time_ns"] = res.exec_time_ns

    out = np.empty((K, M_COLS), dtype=np.float32)
    for core in range(N_CORES):
        kc, mc = divmod(core, MC)
        out[kc * K_LOC : (kc + 1) * K_LOC, mc * M_LOC : (mc + 1) * M_LOC] = (
            _extract_out(res.results[core]["out"])
        )
    return out


# revision 40
# speedup vs baseline: 1.1500x; 1.0475x over previous
"""Trainium2 Bass kernel for nn_ComputeDistances (vq_codebook).

dist[k, m] = || X @ (M[:, m] - c_k) ||_2,  X:[4096,512], M:[512,4096], C:[2048,512]

Reformulated via the Gram matrix G = X^T X (512x512):
    dist^2[k, m] = m^T G m  -  2 c_k^T G m  +  c_k^T G c_k

Sharding: 8 cores as a 2(K) x 4(m) grid; each core computes its
[1024, 1024] output slab independently (no collectives).

All heavy matmuls are fp8e4 DoubleRow (2 fp8 rows per PE pass: a
contraction-512 product needs 2 instructions instead of 4). Measured on
HW: one FD-512 DR matmul streams in ~216ns with LDWEIGHTS hidden.

fp8 range/precision handling:
  - G's diagonal (~4096) would dominate fp8 quantization error, so stage
    A subtracts 4096*I on the PE (one (-64I)^T(64I) matmul per diag
    block) and stages B/B2 restore it with a +128*m-hat correction
    DoubleRow matmul (idp = [128*I; 0] pairs) inside each PSUM group.
  - H = G@M is cast to fp8 as H/256 on the ACT engine; sqXM reduces
    hf8 .* m8 on the Pool engine (all SBUF - GPSIMD cannot touch PSUM),
    sqXC reduces the f32 PSUM on DVE.
  - Host-side rows (nam/nac) cancel the per-query component of the fp8
    rounding of m and c.
  - sqXM/sqXC fold into stage C's PSUM via one contraction-2 matmul.

Scheduling against the HAM clock-gate: the PE must stay busy or the
clock drops to 1.2GHz and stays there. Stage A is upper-triangular
(mirrored via fp8 PE transposes), and stage C is split by m-halves:
C(s=0) interleaves with B(s=1) so the sqrt/DMA stream of the first half
hides under matmul work. All output DMAs issue from the SP queue - a
dma_start costs ~600ns of sequencer time and must not serialize with
the ACT sqrts.

Scale ledger (P* = PSUM value):
  A:  P_G  = G - 4096 I          gxx8 = P_G/32           (fp8)
  B:  P_H  = gxx8@m8 + 128 m8 = H/32
      hf8  = P_H/8 = H/256 (ACT)   p16 = hf8 .* m8 = Hm/256 (Pool)
      sqm  = ones^T p16 = sqXM/256   wwt1 = sqm + nam (via tiny DMA)
  B2: P_W  = gxx8@c8 + 128 c8 = GC2/32   (c8 = -2C^T)
      q16  = P_W .* c8 = c(Gc)/8 (DVE)
      sqc  = (ones/32)^T q16 = sqXC/256  vvt0 = sqc + nac
  C:  P_D  = c8^T @ hf8 + vvt^T wwt = dist^2/256
      out  = Sqrt(256 * P_D)  (ACT, fp16; host upcasts to f32)
"""

import os
import numpy as np

N, D, M_COLS, K = 4096, 512, 4096, 2048
N_CORES = 8
KC, MC = 2, 4  # core grid: K-split x M-split
K_LOC, M_LOC = K // KC, M_COLS // MC  # 1024, 1024

P = 128
XT_N = 8           # X tiles of 512 rows (2 DoubleRow groups each)
XT_R = 4           # sub-rows per partition per X tile
DC = D // P        # 4 contraction chunks over D
MS = M_LOC // 512  # 2 m-slices of 512
KS = K_LOC // 512  # 2 k-slices of 512
KT = K_LOC // P    # 8 k-tiles
WARM_MMS = 28

_compiled = {}


def _build_nc():
    import concourse.mybir as mybir
    import concourse.tile as tile
    from concourse import bacc
    from concourse.masks import make_identity

    f32 = mybir.dt.float32
    f16 = mybir.dt.float16
    bf16 = mybir.dt.bfloat16
    f8 = mybir.dt.float8e4
    DR = mybir.MatmulPerfMode.DoubleRow
    MULT = mybir.AluOpType.mult
    ADD = mybir.AluOpType.add

    nc = bacc.Bacc("TRN2", target_bir_lowering=False, debug=False)

    # host-packed flat layouts: one contiguous span per partition
    x_d = nc.dram_tensor("x", [P, XT_N * XT_R * D], f8, kind="ExternalInput")
    m_d = nc.dram_tensor("m8", [P, DC * M_LOC], f8, kind="ExternalInput")
    c_d = nc.dram_tensor("c8", [P, DC * K_LOC], f8, kind="ExternalInput")
    nam_d = nc.dram_tensor("nam", [1, M_LOC], f16, kind="ExternalInput")
    nac_d = nc.dram_tensor("nac", [1, K_LOC], f16, kind="ExternalInput")
    o_d = nc.dram_tensor("out", [K_LOC, M_LOC], f16, kind="ExternalOutput")

    with tile.TileContext(nc) as tc:
        with (
            tc.tile_pool(name="xp", bufs=1) as xp,
            tc.tile_pool(name="res", bufs=1) as res,
            tc.tile_pool(name="wk", bufs=1) as wk,
            tc.tile_pool(name="op", bufs=6) as op,
            tc.tile_pool(name="psA", bufs=4, space="PSUM") as psA,
            tc.tile_pool(name="psS", bufs=1, space="PSUM") as psS,
        ):
            # ---- PE warmup: tiny bf16 matmuls on zero tiles (no input deps) ----
            wl = res.tile([P, 1], bf16, tag="wl")
            wz = res.tile([P, P], bf16, tag="wz")
            nc.vector.memset(wl[:], 0.0)
            nc.vector.memset(wz[:], 0.0)
            wps = psS.tile([1, P], mybir.dt.float32, tag="sqm0")
            for _ in range(WARM_MMS):
                nc.tensor.matmul(wps[:], wl[:], wz[:], start=True, stop=True)

            # ---- input loads: X on both HWDGE queues, then m8/c8 ----
            dma_engs = [nc.sync, nc.scalar]
            xq = []
            for g in range(XT_N):
                t = xp.tile([P, XT_R, D], f8, tag=f"xq{g}", name=f"xq{g}")
                dma_engs[g % 2].dma_start(
                    t[:], x_d.ap()[:, g * XT_R * D : (g + 1) * XT_R * D]
                )
                xq.append(t)
            # m/c inputs follow X on the HWDGE queues (not needed until ~23us)
            ms8 = res.tile([P, DC, M_LOC], f8, tag="ms8")
            ct8 = res.tile([P, DC, K_LOC], f8, tag="ct8")
            nc.scalar.dma_start(ct8[:], c_d.ap())
            nc.sync.dma_start(ms8[:], m_d.ap())
            nam = res.tile([1, M_LOC], f16, tag="nam")
            nac = res.tile([1, K_LOC], f16, tag="nac")
            nc.sync.dma_start(nam[:], nam_d.ap())
            nc.scalar.dma_start(nac[:], nac_d.ap())

            # ---- constants ----
            ones16 = res.tile([P, P], f16, tag="ones16")
            nc.vector.memset(ones16[:], 1.0)
            ones32 = res.tile([P, P], f16, tag="ones32")
            nc.vector.memset(ones32[:], 1.0 / 32.0)
            idf = res.tile([P, P], f32, tag="idf")
            make_identity(nc, idf[:])
            id8 = res.tile([P, P], f8, tag="id8")
            nc.vector.tensor_scalar_mul(id8[:], idf[:], 1.0)
            # rank-2 fold tiles: vvt = [1;sqxc], wwt = [sqxm;1]. Vector engines
            # cannot write partition 1, so the DMA-filled row is the EARLY
            # sqXC one (latency hides under stage B); sqXM lands in wwt row 0
            # via a plain DVE write right before its first use.
            vvt = res.tile([2, K_LOC], f16, tag="vvt")
            nc.vector.memset(vvt[:], 1.0)
            wwt = res.tile([2, M_LOC], f16, tag="wwt")
            nc.vector.memset(wwt[:], 1.0)
            sqxc16 = res.tile([1, K_LOC], f16, tag="sqxc16")

            # resident intermediates
            gxx8 = res.tile([P, DC, D], f8, tag="gxx8")    # (G - 4096 I)/32
            hf8 = res.tile([P, DC, M_LOC], f8, tag="hf8")  # H/256
            q16t = res.tile([P, DC, K_LOC], f16, tag="q16t")   # P_W .* c8
            p16t = res.tile([P, DC, M_LOC], f16, tag="p16t")   # hf8 .* m8

            # ---- stage A: upper-tri G = X^T X - 4096 I ----
            ptags = ["sqm0", "sqm1", "sqc0", "sqc1"]
            pgs = [
                psS.tile([P, 512 - 128 * t], mybir.dt.float32, tag=ptags[t],
                         name=f"pgA{t}")
                for t in range(DC)
            ]
            first = True
            for g in range(XT_N):
                xt = xq[g]
                for f in range(0, XT_R, 2):
                    for t in range(DC):
                        nc.tensor.matmul(
                            pgs[t][:],
                            xt[:, f : f + 2, t * P : (t + 1) * P],
                            xt[:, f : f + 2, t * P :],
                            start=first,
                            stop=(g == XT_N - 1 and f == XT_R - 2),
                            perf_mode=DR,
                        )
                    first = False
            # diag copies: DVE + ACT split
            for c in range(DC):
                eng = nc.vector.tensor_scalar_mul if c % 2 == 0 else nc.scalar.mul
                eng(gxx8[:, c, c * P :], pgs[c][:], 1.0 / 32.0)

            def emit_mirrors():
                # fp8 transpose mode requires output element step of 2
                for t in range(DC):
                    for c in range(t + 1, DC):
                        tp = psA.tile([P, P, 2], f8, tag="ph")
                        nc.tensor.transpose(
                            tp[:, :, 0], gxx8[:, t, c * P : (c + 1) * P], id8[:]
                        )
                        nc.vector.tensor_copy(gxx8[:, c, t * P : (t + 1) * P],
                                              tp[:, :, 0])

            sqc = [
                psS.tile([P, 512], mybir.dt.float32, tag=f"sqc{s}", name=f"sqc{s}")
                for s in range(KS)
            ]
            sqm = [
                psS.tile([P, 512], mybir.dt.float32, tag=f"sqm{s}", name=f"sqm{s}")
                for s in range(MS)
            ]

            def emit_B2(t, s):
                ph = psA.tile([P, 512], mybir.dt.float32, tag="ph")
                for j in range(2):
                    nc.tensor.matmul(
                        ph[:],
                        gxx8[:, 2 * j : 2 * j + 2, t * P : (t + 1) * P],
                        ct8[:, 2 * j : 2 * j + 2, s * 512 : (s + 1) * 512],
                        start=(j == 0),
                        stop=(j == 1),
                        perf_mode=DR,
                    )
                nc.vector.tensor_tensor(
                    q16t[:, t, s * 512 : (s + 1) * 512],
                    ph[:],
                    ct8[:, t, s * 512 : (s + 1) * 512],
                    MULT,
                )

            def emit_B(t, s):
                ph = psA.tile([P, 512], mybir.dt.float32, tag="ph")
                for j in range(2):
                    nc.tensor.matmul(
                        ph[:],
                        gxx8[:, 2 * j : 2 * j + 2, t * P : (t + 1) * P],
                        ms8[:, 2 * j : 2 * j + 2, s * 512 : (s + 1) * 512],
                        start=(j == 0),
                        stop=(j == 1),
                        perf_mode=DR,
                    )
                # s=1 casts on DVE: the ACT queue is sqrt-busy during C(s=0)
                if s == 0:
                    nc.scalar.mul(hf8[:, t, s * 512 : (s + 1) * 512], ph[:], 0.125)
                else:
                    nc.vector.tensor_scalar_mul(
                        hf8[:, t, s * 512 : (s + 1) * 512], ph[:], 0.125
                    )
                nc.gpsimd.tensor_tensor(
                    p16t[:, t, s * 512 : (s + 1) * 512],
                    hf8[:, t, s * 512 : (s + 1) * 512],
                    ms8[:, t, s * 512 : (s + 1) * 512],
                    MULT,
                )

            def emit_C(kt, s):
                pgc = psA.tile([P, 512], mybir.dt.float32, tag="ph")
                for j in range(2):
                    nc.tensor.matmul(
                        pgc[:],
                        ct8[:, 2 * j : 2 * j + 2, kt * P : (kt + 1) * P],
                        hf8[:, 2 * j : 2 * j + 2, s * 512 : (s + 1) * 512],
                        start=(j == 0),
                        stop=False,
                        perf_mode=DR,
                    )
                nc.tensor.matmul(
                    pgc[:],
                    vvt[:, kt * P : (kt + 1) * P],
                    wwt[:, s * 512 : (s + 1) * 512],
                    start=False,
                    stop=True,
                )
                ob = op.tile([P, 512], f16, tag="ob")
                nc.scalar.activation(
                    ob[:], pgc[:], mybir.ActivationFunctionType.Sqrt, scale=256.0
                )
                nc.sync.dma_start(
                    o_d.ap()[kt * P : (kt + 1) * P, s * 512 : (s + 1) * 512],
                    ob[:],
                )

            # ---- B2 (t=3 first, mirrors overlap), sqc reduction, vvt ----
            emit_B2(DC - 1, 0)
            emit_B2(DC - 1, 1)
            emit_mirrors()
            for t in range(DC - 2, -1, -1):
                emit_B2(t, 0)
                emit_B2(t, 1)
            for idx, t in enumerate(range(DC - 1, -1, -1)):
                for s in range(KS):
                    nc.tensor.matmul(
                        sqc[s][:],
                        ones32[:],
                        q16t[:, t, s * 512 : (s + 1) * 512],
                        start=(idx == 0),
                        stop=(idx == DC - 1),
                    )
            for s in range(KS):
                nc.vector.tensor_tensor(
                    sqxc16[0:1, s * 512 : (s + 1) * 512],
                    sqc[s][0:1, :],
                    nac[0:1, s * 512 : (s + 1) * 512],
                    ADD,
                )
            nc.sync.dma_start(vvt[1:2, :], sqxc16[0:1, :])

            # ---- B(s=0), sqm[0], wwt half ----
            for t in range(DC - 1, -1, -1):
                emit_B(t, 0)
            for idx, t in enumerate(range(DC - 1, -1, -1)):
                nc.tensor.matmul(
                    sqm[0][:],
                    ones16[:],
                    p16t[:, t, 0:512],
                    start=(idx == 0),
                    stop=(idx == DC - 1),
                )
            nc.vector.tensor_tensor(
                wwt[0:1, 0:512], sqm[0][0:1, :], nam[0:1, 0:512], ADD
            )

            # ---- C(s=0) interleaved with B(s=1), front-loaded, to keep the
            # PE dense through the sqrt stream ----
            for kt in range(KT):
                emit_C(kt, 0)
                if 1 <= kt <= DC:
                    emit_B(DC - kt, 1)
                elif kt == DC + 1:
                    for idx, t in enumerate(range(DC - 1, -1, -1)):
                        nc.tensor.matmul(
                            sqm[1][:],
                            ones16[:],
                            p16t[:, t, 512:1024],
                            start=(idx == 0),
                            stop=(idx == DC - 1),
                        )
                elif kt == DC + 2:
                    nc.vector.tensor_tensor(
                        wwt[0:1, 512:1024], sqm[1][0:1, :],
                        nam[0:1, 512:1024], ADD
                    )

            # ---- C(s=1) ----
            for kt in range(KT):
                emit_C(kt, 1)

    nc.compile()
    return nc


def _get_nc():
    if "nc" not in _compiled:
        _compiled["nc"] = _build_nc()
    return _compiled["nc"]


def _make_in_maps(X, Mf, C):
    import concourse.mybir as mybir

    np8 = mybir.dt.np(mybir.dt.float8e4)
    x8 = np.ascontiguousarray(X).astype(np8)
    # pack rows g*(P*XT_R) + p*XT_R + f -> [p, (g*XT_R + f)*512 + d]
    xp8 = np.ascontiguousarray(
        x8.reshape(XT_N, P, XT_R, D).transpose(1, 0, 2, 3)
        .reshape(P, XT_N * XT_R * D)
    )
    in_maps = []
    for core in range(N_CORES):
        kc, mc = divmod(core, MC)
        Mslab = Mf[:, mc * M_LOC : (mc + 1) * M_LOC]
        Cslab = C[kc * K_LOC : (kc + 1) * K_LOC, :]
        m8 = np.ascontiguousarray(Mslab).astype(np8)
        c8 = np.ascontiguousarray(-2.0 * Cslab.T).astype(np8)
        # pack rows c*128 + p -> [p, c*cols + j]
        m8p = np.ascontiguousarray(
            m8.reshape(DC, P, M_LOC).transpose(1, 0, 2).reshape(P, DC * M_LOC)
        )
        c8p = np.ascontiguousarray(
            c8.reshape(DC, P, K_LOC).transpose(1, 0, 2).reshape(P, DC * K_LOC)
        )
        # corrections for the Gram-diagonal term: computed dist^2 uses the
        # fp8-rounded m-hat/c-hat and the fp8-rounded Gram diagonal; subtract
        # gdd*(2<v,dv>+|dv|^2) + sum(v-hat^2 * Edd) per query. gdd/Edd come
        # from column norms of the fp8 X (cheap input statistics).
        gdd = np.einsum("nd,nd->d", x8.astype(np.float32), x8.astype(np.float32))
        Edd = 32.0 * (gdd / 32.0).astype(np8).astype(np.float32) - gdd
        dmv = m8.astype(np.float32) - Mslab
        dcv = c8.astype(np.float32) / -2.0 - Cslab.T
        mh = m8.astype(np.float32)
        ch = c8.astype(np.float32) / -2.0
        am = (2.0 * np.einsum("dm,dm,d->m", Mslab, dmv, gdd)
              + np.einsum("dm,dm,d->m", dmv, dmv, gdd)
              + np.einsum("dm,dm,d->m", mh, mh, Edd))
        ac = (2.0 * np.einsum("dk,dk,d->k", Cslab.T, dcv, gdd)
              + np.einsum("dk,dk,d->k", dcv, dcv, gdd)
              + np.einsum("dk,dk,d->k", ch, ch, Edd))
        nam = np.ascontiguousarray(-am[None, :] / 256.0).astype(np.float16)
        nac = np.ascontiguousarray(-ac[None, :] / 256.0).astype(np.float16)
        in_maps.append({"x": xp8, "m8": m8p, "c8": c8p, "nam": nam, "nac": nac})
    return in_maps


def _extract_out(raw):
    return np.asarray(raw).astype(np.float32)


def kernel(in_activations, M, centroids):
    from concourse import bass_utils

    X = np.asarray(in_activations, dtype=np.float32)
    Mf = np.asarray(M, dtype=np.float32)
    C = np.asarray(centroids, dtype=np.float32)

    nc = _get_nc()
    in_maps = _make_in_maps(X, Mf, C)

    res = bass_utils.run_bass_kernel_spmd(
        nc,
        in_maps,
        core_ids=list(range(N_CORES)),
        trace=bool(int(os.environ.get("KERNEL_TRACE", "0"))),
    )
    if res.exec_time_ns is not None:
        print(f"HW exec time: {res.exec_time_ns} ns")
        _compiled["exec_time_ns"] = res.exec_time_ns

    out = np.empty((K, M_COLS), dtype=np.float32)
    for core in range(N_CORES):
        kc, mc = divmod(core, MC)
        out[kc * K_LOC : (kc + 1) * K_LOC, mc * M_LOC : (mc + 1) * M_LOC] = (
            _extract_out(res.results[core]["out"])
        )
    return out
